# revision 58
# baseline (speedup 1.0000x reference)
"""Trainium2 Bass kernel for nn_AutoregressiveFlowLayer (v22).

Computes, for batch x [B, D] and R ragged regions (padded to RMAX):
    xg   = x[:, idx] * valid                       [B, R, RMAX]
    h1   = relu(xg @ (W1*M1))                      [B, R, 128]
    h2   = relu(h1 @ (W2*M2))                      [B, R, 128]
    out  = h2 @ (Wout*Mout) -> (shift, log_s)      [B, R, RMAX, 2]
    u    = (xg - shift) * exp(-log_s)
    ll   = sum(valid * (-0.5 u^2 - 0.5 log(2pi) - log_s), -1)   [B, R, 1]

Sharding: data-parallel over batch across 8 NeuronCores; weights replicated.
idx/valid are baked into the compiled program (recompiled if they change).

Final design (v33, 82.3-84.3us measured vs the 102.6us v20 baseline;
all changes validated step-by-step on hardware traces):
  - PAIR PSUM tiles [128,1024]: the 8 per-step [128,512] relu
    evacuations of v20 become 4 FD=1024 ops (amortizes the ~290-cycle
    per-op engine overhead).  php = 2 pair bufs whose 4-alloc/step
    rotation pins lane A to buf0 / lane B to buf1; pl3 = 1 pair
    [logs|shift]; pll = 2 single banks.  4+2+2 = 8 banks.
  - SOFTWARE PIPELINING of the whole tail: exp/sub/dd/q of step k-1
    are emitted at the TOP of iteration k (their inputs are a full
    step old), and L1(k+1)+relu1(k+1) are emitted before L3(k).  The
    in-order engine queues therefore never couple one step's latency
    chain into the next step's start - this was worth ~35us: without
    it the pipeline settles into a ~5-6us/step lock-step equilibrium,
    with it 3.66us/step (DVE-bound: relu1B+relu2B+sub+dd = 3.5us).
  - q = 0.5 u^2 computed as (d*d) * exp(-2*logs - ln2): dd = d^2 on
    DVE (bf16 SBUF tensor_tensor, 2x mode, 327ns) runs concurrently
    with ACT's exp, so GPSIMD's only op (q = dd*E2) starts half a step
    earlier.  Keeping dd OFF GPSIMD is load-bearing: with dd+q both on
    GPSIMD the deferred reduce matmuls reach the PE queue head before
    q is ready and the kernel falls into the slow equilibrium.
  - the ll reduce is deferred FIVE steps and fully matmul-ized:
    ll4 = -(v.q) - sum_k(v*logs), the logs part via host-precomputed
    wv_r = Wlg_r @ v_r as 4 accumulating matmuls at DISTINCT col
    groups (concurrent on the PE array); the q matmul is M=128 with
    zero weight cols (initializes the whole pll bank at window start).
    4 steps share one pll bank at rows 32i+j; one Identity+bias
    copy-out on ACT and ONE 256KB DMA per window, raw layout undone in
    host _assemble.
  - PE HAM warm-up: 8 dummy matmuls depending only on a memset run
    during the DMA-fill window, so real matmuls start at 2.4GHz.
  - input DMAs ordered by need-time (sync-queue issue is ~0.6us each,
    so order is arrival time); reduce backlog drained early near the
    end to shorten the epilogue.
  Remaining time: ~5.5us framework preamble + ~10us pipeline ramp
  (DMA receipt-bound) + 16 x 3.66us steady + ~3us drain + ~8.5us
  framework postamble (semaphore-file reset) + ~2.5us final DMA
  receipt.
"""

import sys

import numpy as np

_TRN_REPO = "/opt/trn_rl_repo"
if _TRN_REPO not in sys.path:
    sys.path.insert(0, _TRN_REPO)

D = 1024
R = 32
RMAX = 32
H1 = 128
H2 = 128
B = 8192
NCORES = 8
BC = B // NCORES          # batch per core
NG = R // 4               # 8 groups of 4 regions
BH = 512                  # batch half-tile (one PSUM bank of fp32)
LN2PI = float(np.log(2.0 * np.pi))
EXP_BIAS = float(-np.log(2.0))  # exp(-2*logs + b) = exp(-2*logs)/2

_cache = {}


def _build_program(idx, valid):
    import concourse.mybir as mybir
    import concourse.tile as tile
    from concourse import bacc

    dt = mybir.dt
    AF = mybir.ActivationFunctionType

    nc = bacc.Bacc("TRN2", target_bir_lowering=False, debug=False)

    # ---- DRAM tensors (per-core inputs) ----
    xg_d = nc.dram_tensor("xg", [128, NG * BC], dt.bfloat16, kind="ExternalInput").ap()
    w1 = nc.dram_tensor("w1", [128, NG, 128], dt.bfloat16, kind="ExternalInput").ap()
    w2 = nc.dram_tensor("w2", [128, R, 128], dt.bfloat16, kind="ExternalInput").ap()
    w3 = nc.dram_tensor("w3", [128, R, 64], dt.bfloat16, kind="ExternalInput").ap()
    negv = nc.dram_tensor("negv", [128, 4, NG, 128], dt.bfloat16, kind="ExternalInput").ap()
    wvp = nc.dram_tensor("wvp", [128, 4, R, 32], dt.bfloat16, kind="ExternalInput").ap()
    cb = nc.dram_tensor("cb", [128, 4], dt.float32, kind="ExternalInput").ap()
    # raw per-window ll banks; the host _assemble untangles the
    # (row = 32*i + j) -> (region, batch-half) mapping
    out_d = nc.dram_tensor("out", [4, 128, BH], dt.float32, kind="ExternalOutput").ap()

    from contextlib import ExitStack

    with tile.TileContext(nc) as tc, ExitStack() as ctx:
        singles = ctx.enter_context(tc.tile_pool(name="singles", bufs=1))
        h1pool = ctx.enter_context(tc.tile_pool(name="h1pool", bufs=4))
        # h2 pairs are also read by the 5-step-deferred reduce matmuls
        h2pool = ctx.enter_context(tc.tile_pool(name="h2pool", bufs=14))
        es = ctx.enter_context(tc.tile_pool(name="es", bufs=26))
        # (PSUM pools are entered after the PE warm-up block below so
        # the warm-up can borrow a bank; the layout is php = 2 pair
        # slabs [128,1024] cycling L1A,L1B,L2A,L2B, pl3 = 1 pair
        # [logs|shift] with one-step-deferred tail, pll = 2 banks each
        # collecting 4 steps' ll rows.  4+2+2 = 8 banks.)

        # ---- load constants into SBUF ----
        w1s = singles.tile([128, NG, 128], dt.bfloat16)
        w2s = singles.tile([128, R, 128], dt.bfloat16)
        w3s = singles.tile([128, R, 64], dt.bfloat16)
        negvs = singles.tile([128, 4, NG, 128], dt.bfloat16)
        wvps = singles.tile([128, 4, R, 32], dt.bfloat16)
        cbs = singles.tile([128, 4], dt.float32)

        # gathered ragged inputs (bf16, host-side gather): one tile per
        # group so compute on group g only waits for its own slab.
        xgb = []
        for g in range(NG):
            t = singles.tile([128, 1, BC], dt.bfloat16, tag=f"xgb{g}")
            xgb.append(t)

        # startup-critical slices first, ordered by when the pipeline
        # needs them (each sync-queue dma_start costs ~0.6us of issue
        # time, so order IS arrival time).  Constants for the deferred
        # reduce and the late xg groups go through GPSIMD's SWDGE queue,
        # which is idle during the ramp.
        nc.sync.dma_start(out=xgb[0][:, :, 0:BH], in_=xg_d[:, 0:BH])
        nc.sync.dma_start(out=w1s[:, 0, :], in_=w1[:, 0, :])
        nc.sync.dma_start(out=w2s[:, 0:4, :], in_=w2[:, 0:4, :])
        nc.sync.dma_start(out=w3s[:, 0:4, :], in_=w3[:, 0:4, :])
        nc.sync.dma_start(out=xgb[0][:, :, BH:BC], in_=xg_d[:, BH:BC])
        nc.sync.dma_start(out=xgb[1][:, :, 0:BH], in_=xg_d[:, BC:BC + BH])
        nc.sync.dma_start(out=w1s[:, 1:NG, :], in_=w1[:, 1:NG, :])
        nc.sync.dma_start(out=w2s[:, 4:R, :], in_=w2[:, 4:R, :])
        nc.sync.dma_start(out=xgb[1][:, :, BH:BC], in_=xg_d[:, BC + BH:2 * BC])
        nc.sync.dma_start(out=w3s[:, 4:R, :], in_=w3[:, 4:R, :])
        nc.sync.dma_start(out=xgb[2][:, :, 0:BH], in_=xg_d[:, 2 * BC:2 * BC + BH])
        nc.sync.dma_start(out=xgb[2][:, :, BH:BC], in_=xg_d[:, 2 * BC + BH:3 * BC])
        nc.sync.dma_start(out=xgb[3][:], in_=xg_d[:, 3 * BC:4 * BC])
        nc.sync.dma_start(out=cbs[:], in_=cb)
        nc.sync.dma_start(out=negvs[:], in_=negv)
        nc.sync.dma_start(out=wvps[:], in_=wvp)
        for g in range(4, NG):
            nc.sync.dma_start(out=xgb[g][:], in_=xg_d[:, g * BC:(g + 1) * BC])

        # per-partition constant bias for the exp
        ebias = singles.tile([128, 1], dt.float32)
        nc.vector.memset(ebias[:], EXP_BIAS)

        # warm-load dummies: pull ACT_TABLE_LOAD + Q7 ucode load into the
        # preamble dead time.
        wl0 = singles.tile([1, 1], dt.bfloat16)
        nc.scalar.activation(wl0[:], ebias[0:1, 0:1], AF.Exp)
        wl1 = singles.tile([1, 1], dt.bfloat16)
        nc.gpsimd.tensor_mul(wl1[:], ebias[0:1, 0:1], ebias[0:1, 0:1])

        # PE HAM warm-up: ~3.5us of dummy matmuls that depend only on
        # memsets (not on any DMA), so they run while the input DMAs
        # are still in flight and un-throttle the PE clock (K=4/8 ->
        # 8/8) before the first real matmul.
        wlh = singles.tile([1, 1], dt.bfloat16)
        nc.vector.memset(wlh[:], 0.0)
        wrhs = singles.tile([1, BH], dt.bfloat16)
        nc.vector.memset(wrhs[:], 0.0)
        with tc.tile_pool(name="warm", bufs=1, space="PSUM") as wpool:
            wps = wpool.tile([1, BH], dt.float32)
            for _ in range(8):
                nc.tensor.matmul(out=wps[:], lhsT=wlh[:], rhs=wrhs[:],
                                 start=True, stop=True)

        php = ctx.enter_context(tc.tile_pool(name="php", bufs=2, space="PSUM"))
        pl3 = ctx.enter_context(tc.tile_pool(name="pl3", bufs=1, space="PSUM"))
        pll = ctx.enter_context(tc.tile_pool(name="pll", bufs=2, space="PSUM"))

        nh = BC // BH  # halves per core
        nsteps = NG * nh

        def relu(on_act, dst, src):
            if on_act:
                nc.scalar.activation(dst, src, AF.Relu)
            else:
                nc.vector.tensor_scalar_max(dst, src, 0.0)

        # deferred reduce of step `prev`: accumulating matmuls
        # ll4 = -(v.q) - sum_k v*logs into the shared pll bank at
        # partition offset 32*(s%4) (M=32, unused weight cols are zero
        # so the whole bank stays initialized).  The logs part uses the
        # host-precomputed wv_r = (Wlg_r @ v_r): sum_k v*logs = h2.wv,
        # one matmul per region - no logs evacuation op needed at all.
        # Every 4 steps: one ACT Identity+bias copy-out + 4 small DMAs.
        state = {"ll": None}

        def emit_reduce(prev):
            # ll row layout within a window: row 32*i + j holds region
            # 4g+i of window position j.  The q matmul is M=128 with
            # only cols 32i+j nonzero (j==0 initializes the whole bank,
            # j>0 accumulate); the 4 wv matmuls hit DISTINCT col groups
            # (0,32i) so they run concurrently on the PE array.
            qt, h2pair, s = prev
            g = s // nh
            j = s % 4
            if j == 0:
                state["ll"] = pll.tile([128, BH], dt.float32, tag="ll",
                                       name="llt")
            # the M=128 q matmul spans the whole bank, so it carries the
            # group start (j==0, emitted first) and stop (j==3, emitted
            # last) flags; the M=32 wv matmuls only cover 32-row slices.
            def q_mm(start, stop):
                nc.tensor.matmul(
                    out=state["ll"][:, 0:BH], lhsT=negvs[:, j, g, :],
                    rhs=qt[:], start=start, stop=stop,
                    tile_position=(0, 0),
                )
            if j < 3:
                q_mm(j == 0, False)
            for i in range(4):
                nc.tensor.matmul(
                    out=state["ll"][32 * i:32 * (i + 1), 0:BH],
                    lhsT=wvps[:, j, 4 * g + i, :],
                    rhs=h2pair[i // 2][:, BH * (i % 2):BH * (i % 2 + 1)],
                    start=False, stop=False,
                    tile_position=(0, 32 * i),
                )
            if j == 3:
                q_mm(False, True)
            if j == 3:
                c = s // 4
                lls = singles.tile([128, BH], dt.float32, tag=f"lls{c}",
                                   name="lls")
                nc.vector.tensor_scalar_add(lls[:], state["ll"][:],
                                            cbs[:, c:c + 1])
                nc.sync.dma_start(out=out_d[c], in_=lls[:])

        # deferred tail of step s: by the time it is emitted (one step
        # later) all its inputs are long computed, so the in-order ACT/
        # DVE queues never stall on it - the queue-order coupling
        # "next step's relus wait this step's exp/sub" disappears.
        def emit_tail(pend):
            l3p, xgbs, s = pend
            lgsl = l3p[:, 0:BH]
            shsl = l3p[:, BH:2 * BH]
            # E2 = exp(-2*logs)/2  (ACT)
            et = es.tile([128, BH], dt.bfloat16, tag="et", name="et")
            nc.scalar.activation(et[:], lgsl, AF.Exp,
                                 bias=ebias[:], scale=-2.0)
            # d = xg - shift  (DVE, PSUM operand)
            dtl = es.tile([128, BH], dt.bfloat16, tag="dt", name="dtl")
            nc.vector.tensor_sub(dtl[:], xgbs, shsl)
            # dd = d^2 on ACT (Square) - DVE is the 96%-busy bound and
            # ACT has slack; keeping dd off GPSIMD remains load-bearing
            # (GPSIMD dd re-creates the slow stall cycle).
            ddt = es.tile([128, BH], dt.bfloat16, tag="ddt", name="ddt")
            nc.scalar.activation(ddt[:], dtl[:], AF.Square)
            qt = es.tile([128, BH], dt.bfloat16, tag="qt", name="qt")
            nc.gpsimd.tensor_mul(qt[:], ddt[:], et[:])
            return qt

        # engine split: True = ACT.  DVE carries sub, ACT carries exp
        # (+ copy-out every 4th step).  Pair relus split 2/2.
        RELU_ACT = (True, False, True, False)

        def emit_L1(k):
            g, h = k // nh, k % nh
            xgbs = xgb[g][:, 0, h * BH:(h + 1) * BH]
            l1p = [php.tile([128, 2 * BH], dt.float32, tag="ph", name="l1p")
                   for _ in range(2)]
            for j in range(4):
                nc.tensor.matmul(
                    out=l1p[j // 2][:, BH * (j % 2):BH * (j % 2 + 1)],
                    lhsT=w1s[32 * j:32 * (j + 1), g, :],
                    rhs=xgbs[32 * j:32 * (j + 1), :],
                    start=True, stop=True,
                    tile_position=(32 * j, 0),
                )
            h1sb = []
            for p in range(2):
                ht = h1pool.tile([128, 2 * BH], dt.bfloat16, tag="hsb",
                                 name="h1t")
                relu(RELU_ACT[p], ht[:], l1p[p][:])
                h1sb.append(ht)
            return h1sb, xgbs

        def emit_L2(k, h1sb):
            g = k // nh
            l2p = [php.tile([128, 2 * BH], dt.float32, tag="ph", name="l2p")
                   for _ in range(2)]
            for j in range(4):
                nc.tensor.matmul(
                    out=l2p[j // 2][:, BH * (j % 2):BH * (j % 2 + 1)],
                    lhsT=w2s[:, 4 * g + j, :],
                    rhs=h1sb[j // 2][:, BH * (j % 2):BH * (j % 2 + 1)],
                    start=True, stop=True,
                    tile_position=(0, 0),
                )
            h2sb = []
            for p in range(2):
                ht = h2pool.tile([128, 2 * BH], dt.bfloat16, tag="hsb",
                                 name="h2t")
                relu(RELU_ACT[2 + p], ht[:], l2p[p][:])
                h2sb.append(ht)
            return h2sb

        def emit_L3(k, h2sb):
            # ONE pair slab [logs | shift], col-tiled M=32 matmuls.
            # Its only reader is the one-step-deferred tail, so these
            # matmuls are off the latency-critical path.
            g = k // nh
            l3p = pl3.tile([128, 2 * BH], dt.float32, tag="l3", name="l3p")
            for j in range(4):
                nc.tensor.matmul(
                    out=l3p[32 * j:32 * (j + 1), 0:BH],
                    lhsT=w3s[:, 4 * g + j, 32:64],
                    rhs=h2sb[j // 2][:, BH * (j % 2):BH * (j % 2 + 1)],
                    start=True, stop=True,
                    tile_position=(0, 32 * j),
                )
            for j in range(4):
                nc.tensor.matmul(
                    out=l3p[32 * j:32 * (j + 1), BH:2 * BH],
                    lhsT=w3s[:, 4 * g + j, 0:32],
                    rhs=h2sb[j // 2][:, BH * (j % 2):BH * (j % 2 + 1)],
                    start=True, stop=True,
                    tile_position=(0, 32 * j),
                )
            return l3p

        # Software-pipelined emission: per iteration k the engine queues
        # receive [L2(k)+relu2(k), tail(k-1), L1(k+1)+relu1(k+1), L3(k),
        # reduce(k-3)].  Every tail/reduce op's inputs are computed at
        # least a step earlier, so the in-order queues never couple one
        # step's latency chain into the next step's start.
        pend = None    # step whose tail is not yet emitted
        prevs = []     # steps whose reduce is not yet emitted
        h2keep = {}    # step -> h2 pair tiles (read by its reduce)
        h1_cur, xgbs_cur = emit_L1(0)
        for step in range(nsteps):
            # tail of step-1 FIRST: its inputs are long ready, so ACT/
            # DVE execute it while this step's L2 matmuls are still in
            # flight instead of idling in front of relu2.
            if pend is not None:
                qt = emit_tail(pend)
                prevs.append((qt, pend[2]))
            h2sb = emit_L2(step, h1_cur)
            if step + 1 < nsteps:
                h1_next = emit_L1(step + 1)
            l3p = emit_L3(step, h2sb)
            if len(prevs) == 5:
                qt, s = prevs.pop(0)
                emit_reduce((qt, h2keep[s], s))
            # drain the deferral backlog early near the end so the
            # post-loop tail is short
            if step >= nsteps - 3 and prevs:
                qt, s = prevs.pop(0)
                emit_reduce((qt, h2keep[s], s))
            h2keep[step] = h2sb
            pend = (l3p, xgbs_cur, step)
            if step + 1 < nsteps:
                h1_cur, xgbs_cur = h1_next

        qt = emit_tail(pend)
        prevs.append((qt, pend[2]))
        for qt, s in prevs:
            emit_reduce((qt, h2keep[s], s))

    nc.compile()
    return nc


def _host_prep(inputs, W1, W2, Wout, idx, valid, M1, M2, Mout):
    import ml_dtypes

    bf16 = ml_dtypes.bfloat16
    f32 = np.float32

    idx = np.asarray(idx)
    valid = np.asarray(valid)
    vf = valid.astype(f32)                                  # [R, RMAX]
    Wm1 = (np.asarray(W1) * np.asarray(M1)).astype(f32)     # [R, 32, 128]
    Wm2 = (np.asarray(W2) * np.asarray(M2)).astype(f32)     # [R, 128, 128]
    Wm3 = (np.asarray(Wout) * np.asarray(Mout)).astype(f32)  # [R, 128, 64]
    Wsh = Wm3[:, :, 0::2]                                   # [R, 128, 32]
    Wlg = Wm3[:, :, 1::2]                                   # [R, 128, 32]

    w1 = np.zeros((128, NG, 128), f32)
    for g in range(NG):
        for j in range(4):
            w1[32 * j:32 * (j + 1), g, :] = Wm1[4 * g + j]
    w1 = w1.astype(bf16)
    w2 = np.ascontiguousarray(Wm2.transpose(1, 0, 2)).astype(bf16)  # [128,R,128]
    w3 = np.concatenate([Wsh, Wlg], axis=2)                 # [R, 128, 64]
    w3 = np.ascontiguousarray(w3.transpose(1, 0, 2)).astype(bf16)   # [128,R,64]

    # ll row layout: row 32*i + j = region 4g+i of window position j
    # (step s = 4c + j, g = 2c + j//2).
    # negv[:, j, g, :]: M=128 lhsT for the q reduce; nonzero entries
    # lhsT[32i+ii, 32i+j] = -v[4g+i][ii].
    negv = np.zeros((128, 4, NG, 128), f32)
    for j in range(4):
        for g in range(NG):
            for i in range(4):
                negv[32 * i:32 * (i + 1), j, g, 32 * i + j] = -vf[4 * g + i]
    negv = negv.astype(bf16)

    # wvp[:, j, r, j] = -(Wlg_r @ v_r): the reduce matmul computes
    # -sum_k v*logs for region r as h2_r . wv_r into ll row 32(r%4)+j.
    wvpv = np.zeros((128, 4, R, 32), f32)
    for j in range(4):
        for r in range(R):
            wvpv[:, j, r, j] = -(Wlg[r] @ vf[r])
    wvpv = wvpv.astype(bf16)

    # cb[32*i + j, c] = -0.5*ln(2pi)*sum(v_r) for region r = 4g+i,
    # g = 2c + j//2; the batched ll copy-out adds it per partition.
    cbv = np.zeros((128, 4), f32)
    for c in range(4):
        for j in range(4):
            gg = 2 * c + j // 2
            for i in range(4):
                cbv[32 * i + j, c] = -0.5 * LN2PI * float(vf[4 * gg + i].sum())

    # host-side ragged gather: partition p of group g holds
    # x[:, idx[4g + p//32, p%32]] * valid, transposed to [feat, batch]
    rows = idx.reshape(NG, 4 * RMAX)                        # [NG, 128]
    vflat = vf.reshape(NG, 4 * RMAX)                        # [NG, 128]
    xT = np.asarray(inputs, dtype=f32).T                    # [D, B]
    xg_full = xT[rows.reshape(-1)] * vflat.reshape(-1, 1)   # [NG*128, B]
    xg_full = xg_full.reshape(NG, 128, B).astype(bf16)

    per_core = []
    for c in range(NCORES):
        sl = xg_full[:, :, c * BC:(c + 1) * BC]             # [NG, 128, BC]
        xg = np.ascontiguousarray(sl.transpose(1, 0, 2)).reshape(128, NG * BC)
        per_core.append({
            "xg": xg,
            "w1": w1, "w2": w2, "w3": w3,
            "negv": negv, "wvp": wvpv, "cb": cbv,
        })
    return per_core


def _get_compiled(idx, valid):
    key = (np.asarray(idx).tobytes(), np.asarray(valid).tobytes())
    if _cache.get("key") != key:
        _cache["key"] = key
        _cache["nc"] = _build_program(np.asarray(idx), np.asarray(valid))
    return _cache["nc"]


def _assemble(results):
    # device ll row layout: out[c, 32*i + j, b] = ll for region 4g+i,
    # batch half h of the core, where g = 2c + j//2 and h = j%2.
    full = np.zeros((B, R), np.float32)
    for core in range(NCORES):
        o = results[core]["out"]                    # [4, 128, BH]
        o4 = o.reshape(4, 4, 32, BH)[:, :, 0:4, :]  # [c, i, j, b]
        for c in range(4):
            for j in range(4):
                g = 2 * c + j // 2
                h = j % 2
                rows = slice(core * BC + h * BH, core * BC + (h + 1) * BH)
                for i in range(4):
                    full[rows, 4 * g + i] = o4[c, i, j, :]
    return full[..., None]


def kernel(inputs, W1, W2, Wout, idx, valid, M1, M2, Mout):
    from concourse import bass_utils

    nc = _get_compiled(idx, valid)
    in_maps = _host_prep(inputs, W1, W2, Wout, idx, valid, M1, M2, Mout)
    res = bass_utils.run_bass_kernel_spmd(nc, in_maps, core_ids=list(range(NCORES)))
    out = _assemble(res.results)
    _cache["last_exec_time_ns"] = res.exec_time_ns
    return out


def kernel_profiled(inputs, W1, W2, Wout, idx, valid, M1, M2, Mout, tmpdir=None):
    """Like kernel() but requests an NTFF trace; returns (out, exec_time_ns)."""
    from concourse import bass_utils

    nc = _get_compiled(idx, valid)
    in_maps = _host_prep(inputs, W1, W2, Wout, idx, valid, M1, M2, Mout)
    res = bass_utils.run_bass_kernel_spmd(
        nc, in_maps, core_ids=list(range(NCORES)), trace=True, tmpdir=tmpdir,
    )
    out = _assemble(res.results)
    return out, res.exec_time_ns
